# revision 1
# baseline (speedup 1.0000x reference)
"""GAT 2-layer kernel for trn2, 8-core SPMD.

Sharding: nodes dst-sharded per core; per-core nodes packed into NG groups
(<=128 nodes) in 4 quarters; chunk(src) = owner quarter. Edge slots laid out
as (group, chunk) runs of RUN slots. Layer 1 is gather-free (host supplies
x[src] per slot, transposed). Layer 2 gathers T2 rows by src via dma_gather
after per-quarter AllGathers. Segment softmax-sums via one-hot matmuls.
"""

import numpy as np

import concourse.bacc as bacc
import concourse.bass as bass
import concourse.mybir as mybir
import concourse.tile as tile
from concourse import library_config
from concourse.bass_utils import run_bass_kernel_spmd
from concourse._compat import cdiv

F32 = mybir.dt.float32
F16 = mybir.dt.float16
I16 = mybir.dt.int16

NEG_SLOPE = 0.2
EPS = 1e-16


class Cfg:
    def __init__(self, n_nodes, n_cores, h1, c1, h2, c2, in_dim, ng, run):
        self.N = n_nodes
        self.K = n_cores
        self.NPC = n_nodes // n_cores
        self.H1, self.C1, self.H2, self.C2 = h1, c1, h2, c2
        self.F1 = h1 * c1
        self.F2 = h2 * c2
        self.D = in_dim
        self.NG = ng
        self.QG = ng // 4
        self.RUN = run
        self.RG = run // 128
        self.GRAN_G = 4 * self.RG
        self.SLOTS_G = 4 * run
        self.SLOTS = ng * self.SLOTS_G
        self.ROWS_PC = ng * 128
        self.CR = n_cores * self.QG * 128
        self.NT = 4 * self.CR
        assert self.CR <= 32768, f"chunk rows {self.CR} > 32768"
        self.B = max(1, 1024 // run)
        if self.B > ng:
            self.B = ng
        while ng % self.B:
            self.B -= 1
        self.W = self.B * run
        assert self.W <= 1024
        self.NB = ng // self.B


def _pack(ng, deg_chunk):
    """Pack nodes into ng groups of <=128, balancing 4 per-chunk sums."""
    npc = deg_chunk.shape[0]
    order = np.argsort(-deg_chunk.sum(1), kind="stable")
    counts = np.zeros((ng, 4), np.int64)
    sizes = np.zeros(ng, np.int64)
    group_of = np.zeros(npc, np.int64)
    for n in order:
        d = deg_chunk[n]
        score = (counts + d[None, :]).max(1) * 1000 + sizes
        score[sizes >= 128] = 1 << 60
        g = int(np.argmin(score))
        group_of[n] = g
        counts[g] += d
        sizes[g] += 1
    slot_of = np.zeros(npc, np.int64)
    nxt = np.zeros(ng, np.int64)
    for n in range(npc):
        g = group_of[n]
        slot_of[n] = nxt[g]
        nxt[g] += 1
    return group_of, slot_of, counts


def preprocess(inputs, cfg, verbose=False):
    x = np.asarray(inputs["x"], np.float32)
    ei = np.asarray(inputs["edge_index"])
    labels = np.asarray(inputs["labels"]).astype(np.int64)
    tmask = np.asarray(inputs["train_mask"]).astype(bool)
    W1 = np.asarray(inputs["W1"], np.float32)
    a_s1 = np.asarray(inputs["a_src1"], np.float32)
    a_d1 = np.asarray(inputs["a_dst1"], np.float32)
    b1 = np.asarray(inputs["b1"], np.float32)
    W2 = np.asarray(inputs["W2"], np.float32)
    a_s2 = np.asarray(inputs["a_src2"], np.float32)
    a_d2 = np.asarray(inputs["a_dst2"], np.float32)
    b2 = np.asarray(inputs["b2"], np.float32)

    K, NPC, NG, QG, RUN = cfg.K, cfg.NPC, cfg.NG, cfg.QG, cfg.RUN
    src, dst = ei[0].astype(np.int64), ei[1].astype(np.int64)

    # pass 1: quarters (balance total in-degree per core)
    deg = np.bincount(dst, minlength=cfg.N)
    quarter_of = np.zeros(cfg.N, np.int64)
    for k in range(K):
        nodes = np.arange(k * NPC, (k + 1) * NPC)
        order = nodes[np.argsort(-deg[nodes], kind="stable")]
        qcap = np.full(4, QG * 128, np.int64)
        qload = np.zeros(4, np.float64)
        for n in order:
            q = int(np.argmin(np.where(qcap > 0, qload, np.inf)))
            quarter_of[n] = q
            qcap[q] -= 1
            qload[q] += deg[n]
    chunk_of_src = quarter_of[src]

    # pass 2: groups within quarters
    group_of = np.zeros(cfg.N, np.int64)
    slot_of = np.zeros(cfg.N, np.int64)
    maxrun = 0
    for k in range(K):
        base = k * NPC
        dc = np.zeros((NPC, 4), np.int64)
        m_core = (dst >= base) & (dst < base + NPC)
        np.add.at(dc, (dst[m_core] - base, chunk_of_src[m_core]), 1)
        nodes = np.arange(base, base + NPC)
        for q in range(4):
            qn = nodes[quarter_of[nodes] == q]
            go, so, counts = _pack(QG, dc[qn - base])
            group_of[qn] = q * QG + go
            slot_of[qn] = so
            maxrun = max(maxrun, int(counts.max()))
    if verbose:
        print("pack max (g,c):", maxrun, "RUN:", RUN)
    assert maxrun <= RUN, f"packing overflow: {maxrun} > RUN={RUN}"

    core_of = np.arange(cfg.N) // NPC
    gq = group_of % QG
    row_of = quarter_of * cfg.CR + core_of * (QG * 128) + gq * 128 + slot_of

    x16 = x.astype(np.float16)
    Wc1e = np.concatenate(
        [W1, (W1.reshape(cfg.D, cfg.H1, cfg.C1) * a_s1[None]).sum(-1)], axis=1)
    M1d = (W1.reshape(cfg.D, cfg.H1, cfg.C1) * a_d1[None]).sum(-1)
    Wcat2 = np.concatenate(
        [W2, (W2.reshape(cfg.F1, cfg.H2, cfg.C2) * a_s2[None]).sum(-1),
         (W2.reshape(cfg.F1, cfg.H2, cfg.C2) * a_d2[None]).sum(-1)], axis=1)
    iota = np.tile(np.arange(128, dtype=np.float16)[None, :], (128, 1))
    ident = np.eye(128, dtype=np.float16)

    in_maps = []
    perm_rows = np.zeros((K, NG * 128), np.int64)
    for k in range(K):
        base = k * NPC
        m_core = (dst >= base) & (dst < base + NPC)
        es, ed, ec = src[m_core], dst[m_core], chunk_of_src[m_core]
        eg = group_of[ed]
        okey = (eg * 4 + ec) * (1 << 20) + (row_of[es] >> 20)
        eo = np.lexsort((row_of[es], eg * 4 + ec))
        es, ed, ec, eg = es[eo], ed[eo], ec[eo], eg[eo]

        run_id = eg * 4 + ec
        rc = np.bincount(run_id, minlength=NG * 4)
        starts = np.zeros(NG * 4, np.int64)
        starts[1:] = np.cumsum(rc)[:-1]
        within = np.arange(es.shape[0]) - starts[run_id]
        spos = run_id * RUN + within

        SL = cfg.SLOTS
        slot_src = np.full(SL, -1, np.int64)
        slot_rel = np.full(SL, -1.0, np.float32)
        slot_src[spos] = es
        slot_rel[spos] = slot_of[ed]
        nz = slot_src >= 0

        xg = np.zeros((SL, cfg.D), np.float16)
        xg[nz] = x16[slot_src[nz]]
        xgT = np.ascontiguousarray(xg.T)

        rows = np.zeros(SL, np.int64)
        rows[nz] = row_of[slot_src[nz]]
        chunk_id = np.tile(np.repeat(np.arange(4), RUN), NG)
        loc = rows - chunk_id * cfg.CR
        loc[~nz] = 0
        idx16 = np.zeros((4, cfg.NB, 128, cfg.W // 16), np.int16)
        for c in range(4):
            for b in range(cfg.NB):
                sel = np.zeros(cfg.W, np.int64)
                for gi in range(cfg.B):
                    g = b * cfg.B + gi
                    s0 = (g * 4 + c) * RUN
                    sel[gi * RUN:(gi + 1) * RUN] = loc[s0:s0 + RUN]
                i = np.arange(cfg.W)
                idx16[c, b, i % 16, i // 16] = sel.astype(np.int16)

        dr = slot_rel.reshape(NG, cfg.GRAN_G, 128)
        dstrel = np.ascontiguousarray(dr.transpose(2, 0, 1)).astype(np.float16)

        nodes = np.arange(base, base + NPC)
        pr = np.full(NG * 128, -1, np.int64)
        pr[group_of[nodes] * 128 + slot_of[nodes]] = nodes
        perm_rows[k] = pr
        pv = pr >= 0
        xs = np.zeros((NG * 128, cfg.D), np.float16)
        xs[pv] = x16[pr[pv]]
        xslabT = np.ascontiguousarray(xs.T)

        mk_flat = np.zeros(NG * 128, np.float32)
        oh_flat = np.zeros((NG * 128, cfg.C2), np.float32)
        mk_flat[pv] = tmask[pr[pv]].astype(np.float32)
        oh_flat[np.arange(NG * 128)[pv], labels[pr[pv]]] = 1.0
        mk = np.ascontiguousarray(mk_flat.reshape(NG, 128).T)
        oh = np.ascontiguousarray(
            oh_flat.reshape(NG, 128, cfg.C2).transpose(1, 0, 2))

        in_maps.append({
            "xgT": xgT, "xslabT": xslabT, "dstrel": dstrel, "idx16": idx16,
            "mask": mk, "onehot": np.ascontiguousarray(
                oh.reshape(128, NG * cfg.C2)),
            "Wc1e": Wc1e.astype(np.float16), "M1d": M1d.astype(np.float16),
            "Wcat2": Wcat2.astype(np.float16),
            "b1rep": np.tile(b1[None, :], (128, 1)).astype(np.float32),
            "b2rep": np.tile(b2[None, :], (128, 1)).astype(np.float32),
            "iota": iota, "ident": ident,
        })

    meta = {"cfg": cfg, "perm_rows": perm_rows,
            "mask_sum": float(tmask.sum())}
    return in_maps, meta


def build(cfg):
    NG, QG, RG, GRAN_G = cfg.NG, cfg.QG, cfg.RG, cfg.GRAN_G
    H1, C1, H2, C2 = cfg.H1, cfg.C1, cfg.H2, cfg.C2
    F1, F2, D = cfg.F1, cfg.F2, cfg.D
    E1 = F1 + H1
    E2 = F2 + 2 * H2
    B, W, NB = cfg.B, cfg.W, cfg.NB

    nc = bacc.Bacc("TRN2", target_bir_lowering=False, debug=False,
                   num_devices=cfg.K)
    dt = nc.dram_tensor
    xgT_d = dt("xgT", [D, cfg.SLOTS], F16, kind="ExternalInput")
    xslabT_d = dt("xslabT", [D, NG * 128], F16, kind="ExternalInput")
    dstrel_d = dt("dstrel", [128, NG, GRAN_G], F16, kind="ExternalInput")
    idx16_d = dt("idx16", [4, NB, 128, W // 16], I16, kind="ExternalInput")
    mask_d = dt("mask", [128, NG], F32, kind="ExternalInput")
    oh_d = dt("onehot", [128, NG * C2], F32, kind="ExternalInput")
    wc1e_d = dt("Wc1e", [D, E1], F16, kind="ExternalInput")
    m1d_d = dt("M1d", [D, H1], F16, kind="ExternalInput")
    wcat2_d = dt("Wcat2", [F1, E2], F16, kind="ExternalInput")
    b1_d = dt("b1rep", [128, F1], F32, kind="ExternalInput")
    b2_d = dt("b2rep", [128, C2], F32, kind="ExternalInput")
    iota_d = dt("iota", [128, 128], F16, kind="ExternalInput")
    ident_d = dt("ident", [128, 128], F16, kind="ExternalInput")

    emb1_d = dt("emb1", [NG * 128, F1], F32, kind="ExternalOutput")
    emb2_d = dt("emb2", [NG * 128, C2], F32, kind="ExternalOutput")
    ce_d = dt("ce", [128, 1], F32, kind="ExternalOutput")

    t2slab = dt("t2slab", [NG * 128, 128], F16)
    t2b = dt("t2b", [NG, 128, H2], F16)
    t2full = dt("t2full", [cfg.NT, 128], F16, addr_space="Shared")

    groups_all = [list(range(cfg.K))]

    with tile.TileContext(nc) as tc:
        nc.gpsimd.load_library(library_config.mlp)
        import contextlib
        with contextlib.ExitStack() as ctx:
            con = ctx.enter_context(tc.tile_pool(name="const", bufs=1))
            sb = ctx.enter_context(tc.tile_pool(name="sb", bufs=3))
            sbg = ctx.enter_context(tc.tile_pool(name="sbg", bufs=3))
            psH = ctx.enter_context(tc.tile_pool(name="psH", bufs=2,
                                                 space="PSUM"))
            psT = ctx.enter_context(tc.tile_pool(name="psT", bufs=1,
                                                 space="PSUM"))
            psZ = ctx.enter_context(tc.tile_pool(name="psZ", bufs=1,
                                                 space="PSUM"))
            psM = ctx.enter_context(tc.tile_pool(name="psM", bufs=1,
                                                 space="PSUM"))

            def cload(name, shape, dtype, src):
                t = con.tile(shape, dtype, name=name)
                nc.sync.dma_start(out=t[:], in_=src)
                return t

            wc1e_sb = cload("wc1e_sb", [D, E1], F16, wc1e_d[:])
            m1d_sb = cload("m1d_sb", [D, H1], F16, m1d_d[:])
            wcat2_sb = cload("wcat2_sb", [F1, E2], F16, wcat2_d[:])
            iota_sb = cload("iota_sb", [128, 128], F16, iota_d[:])
            ident_sb = cload("ident_sb", [128, 128], F16, ident_d[:])
            b1_sb = cload("b1_sb", [128, F1], F32, b1_d[:])
            b2_sb = cload("b2_sb", [128, C2], F32, b2_d[:])
            dr_sb = cload("dr_sb", [128, NG, GRAN_G], F16, dstrel_d[:])
            mask_sb = cload("mask_sb", [128, NG], F32, mask_d[:])
            oh_sb = cload("oh_sb", [128, NG * C2], F32, oh_d[:])
            xslab_sb = cload("xslab_sb", [D, NG * 128], F16, xslabT_d[:])
            eluT_sb = con.tile([128, QG * 128], F16, name="eluT_sb")
            acc2 = con.tile([128, NG, F2 + H2], F32, name="acc2")
            emb2_st = con.tile([128, NG, C2], F32, name="emb2_st")
            ce_st = con.tile([128, NG], F32, name="ce_st")

            ag_insts = []
            t2b_w = {}

            # ---------------- layer 1 ----------------
            for g in range(NG):
                q, gq = g // QG, g % QG
                ps_a = psZ.tile([128, H1], F32, tag="psa")
                nc.tensor.matmul(ps_a[:],
                                 lhsT=xslab_sb[:, g * 128:(g + 1) * 128],
                                 rhs=m1d_sb[:], start=True, stop=True)
                a1d_g = sb.tile([128, H1], F16, tag="a1d")
                nc.vector.tensor_copy(out=a1d_g[:], in_=ps_a[:])

                xg_g = sbg.tile([D, GRAN_G * 128], F16, tag="xg")
                nc.sync.dma_start(
                    out=xg_g[:],
                    in_=xgT_d[:, g * cfg.SLOTS_G:(g + 1) * cfg.SLOTS_G])

                h1e = sb.tile([128, GRAN_G, E1], F16, tag="h1e")
                for r in range(GRAN_G):
                    ps_h = psH.tile([128, E1], F32, tag="psh")
                    nc.tensor.matmul(ps_h[:],
                                     lhsT=xg_g[:, r * 128:(r + 1) * 128],
                                     rhs=wc1e_sb[:], start=True, stop=True)
                    nc.vector.tensor_copy(out=h1e[:, r, :], in_=ps_h[:])

                s_all = sb.tile([128, GRAN_G, 128], F16, tag="sall")
                nc.vector.tensor_tensor(
                    out=s_all[:],
                    in0=dr_sb[:, g, :].unsqueeze(2)
                        .to_broadcast([128, GRAN_G, 128]),
                    in1=iota_sb[:].unsqueeze(1)
                        .to_broadcast([128, GRAN_G, 128]),
                    op=mybir.AluOpType.is_equal)

                ps_t = psT.tile([128, GRAN_G * 128], F16, tag="pst")
                for r in range(GRAN_G):
                    nc.tensor.transpose(out=ps_t[:, r * 128:(r + 1) * 128],
                                        in_=s_all[:, r, :],
                                        identity=ident_sb[:])
                st_all = sb.tile([128, GRAN_G * 128], F16, tag="stall")
                nc.vector.tensor_copy(out=st_all[:], in_=ps_t[:])

                ps_z = psZ.tile([128, GRAN_G * H1], F32, tag="psz")
                for r in range(GRAN_G):
                    nc.tensor.matmul(ps_z[:, r * H1:(r + 1) * H1],
                                     lhsT=st_all[:, r * 128:(r + 1) * 128],
                                     rhs=a1d_g[:], start=True, stop=True)

                zst = sb.tile([128, GRAN_G, H1], F32, tag="zst")
                nc.vector.tensor_tensor(
                    out=zst[:],
                    in0=ps_z[:].rearrange("p (g h) -> p g h", h=H1),
                    in1=h1e[:, :, F1:E1], op=mybir.AluOpType.add)
                zl = sb.tile([128, GRAN_G, H1], F32, tag="zl")
                nc.scalar.activation(zl[:], zst[:],
                                     mybir.ActivationFunctionType.Lrelu,
                                     alpha=NEG_SLOPE)
                wst = sb.tile([128, GRAN_G, H1], F16, tag="wst")
                nc.scalar.activation(wst[:], zl[:],
                                     mybir.ActivationFunctionType.Exp)

                mst = sb.tile([128, GRAN_G, F1], F16, tag="mst")
                nc.vector.tensor_tensor(
                    out=mst[:].rearrange("p g (h c) -> p g h c", h=H1),
                    in0=h1e[:, :, 0:F1].rearrange("p g (h c) -> p g h c",
                                                  h=H1),
                    in1=wst[:].unsqueeze(3)
                        .to_broadcast([128, GRAN_G, H1, C1]),
                    op=mybir.AluOpType.mult)

                ps_m = psM.tile([128, E1], F32, tag="psm")
                for r in range(GRAN_G):
                    nc.tensor.matmul(ps_m[:, 0:F1], lhsT=s_all[:, r, :],
                                     rhs=mst[:, r, :],
                                     start=(r == 0), stop=(r == GRAN_G - 1))
                    nc.tensor.matmul(ps_m[:, F1:E1], lhsT=s_all[:, r, :],
                                     rhs=wst[:, r, :],
                                     start=(r == 0), stop=(r == GRAN_G - 1))

                den = sb.tile([128, H1], F32, tag="den")
                nc.vector.tensor_scalar_add(out=den[:], in0=ps_m[:, F1:E1],
                                            scalar1=EPS)
                rec = sb.tile([128, H1], F32, tag="rec")
                nc.vector.reciprocal(out=rec[:], in_=den[:])
                emb1g = sb.tile([128, F1], F32, tag="emb1g")
                nc.vector.tensor_tensor(
                    out=emb1g[:].rearrange("p (h c) -> p h c", h=H1),
                    in0=ps_m[:, 0:F1].rearrange("p (h c) -> p h c", h=H1),
                    in1=rec[:].unsqueeze(2).to_broadcast([128, H1, C1]),
                    op=mybir.AluOpType.mult)
                nc.vector.tensor_tensor(out=emb1g[:], in0=emb1g[:],
                                        in1=b1_sb[:], op=mybir.AluOpType.add)
                nc.sync.dma_start(out=emb1_d[g * 128:(g + 1) * 128, :],
                                  in_=emb1g[:])

                mn = sb.tile([128, F1], F32, tag="mn")
                nc.vector.tensor_scalar_min(out=mn[:], in0=emb1g[:],
                                            scalar1=0.0)
                ex = sb.tile([128, F1], F32, tag="ex")
                nc.scalar.activation(ex[:], mn[:],
                                     mybir.ActivationFunctionType.Exp)
                pos = sb.tile([128, F1], F32, tag="pos")
                nc.vector.tensor_scalar(out=pos[:], in0=emb1g[:], scalar1=0.0,
                                        scalar2=-1.0,
                                        op0=mybir.AluOpType.max,
                                        op1=mybir.AluOpType.add)
                elu16 = sb.tile([128, F1], F16, tag="elu16")
                nc.vector.tensor_tensor(out=elu16[:], in0=pos[:], in1=ex[:],
                                        op=mybir.AluOpType.add)
                ps_e = psT.tile([128, 128], F16, tag="pse")
                nc.tensor.transpose(out=ps_e[:], in_=elu16[:],
                                    identity=ident_sb[:])
                nc.vector.tensor_copy(
                    out=eluT_sb[:, gq * 128:(gq + 1) * 128], in_=ps_e[:])

                if gq == QG - 1:
                    w_insts = []
                    for g2 in range(q * QG, (q + 1) * QG):
                        gq2 = g2 % QG
                        ps_c = psM.tile([128, E2], F32, tag="psc")
                        nc.tensor.matmul(
                            ps_c[:],
                            lhsT=eluT_sb[:, gq2 * 128:(gq2 + 1) * 128],
                            rhs=wcat2_sb[:], start=True, stop=True)
                        t2st = sb.tile([128, 128], F16, tag="t2st")
                        nc.vector.tensor_copy(out=t2st[:, 0:E2], in_=ps_c[:])
                        nc.vector.memset(t2st[:, E2:128], 0)
                        i1 = nc.sync.dma_start(
                            out=t2slab[g2 * 128:(g2 + 1) * 128, :],
                            in_=t2st[:])
                        t2bs = sb.tile([128, H2], F16, tag="t2bs")
                        nc.vector.tensor_copy(out=t2bs[:],
                                              in_=ps_c[:, F2 + H2:E2])
                        i2 = nc.sync.dma_start(out=t2b[g2], in_=t2bs[:])
                        t2b_w[g2] = i2
                        w_insts.append(i1)
                    ag = nc.gpsimd.collective_compute(
                        "AllGather", mybir.AluOpType.bypass,
                        replica_groups=groups_all,
                        ins=[t2slab[q * QG * 128:(q + 1) * QG * 128, :]],
                        outs=[t2full[q * cfg.CR:(q + 1) * cfg.CR, :]])
                    for wi in w_insts:
                        tile.add_dep_helper(ag, wi, reason="t2slab->ag")
                    ag_insts.append(ag)

            # ---------------- layer 2 ----------------
            for c in range(4):
                for b in range(NB):
                    it = sbg.tile([128, W // 16], I16, tag="idx")
                    nc.sync.dma_start(out=it[:], in_=idx16_d[c, b])
                    gth = sbg.tile([128, W // 128, 128], F16, tag="gth")
                    gi = nc.gpsimd.dma_gather(
                        out_ap=gth[:],
                        in_ap=t2full[c * cfg.CR:(c + 1) * cfg.CR, :],
                        idxs_ap=it[:], num_idxs=W, num_idxs_reg=W,
                        elem_size=128)
                    tile.add_dep_helper(gi, ag_insts[c], reason="ag->gather")

                    z2st = sb.tile([128, B * RG, H2], F32, tag="z2st")
                    s2_of = {}
                    for gx in range(B):
                        g = b * B + gx
                        a2d_g = sb.tile([128, H2], F16, tag="a2d")
                        ld = nc.sync.dma_start(out=a2d_g[:], in_=t2b[g])
                        tile.add_dep_helper(ld, t2b_w[g], reason="t2b->a2d")

                        s2 = sb.tile([128, RG, 128], F16, tag="s2")
                        nc.vector.tensor_tensor(
                            out=s2[:],
                            in0=dr_sb[:, g, c * RG:(c + 1) * RG].unsqueeze(2)
                                .to_broadcast([128, RG, 128]),
                            in1=iota_sb[:].unsqueeze(1)
                                .to_broadcast([128, RG, 128]),
                            op=mybir.AluOpType.is_equal)
                        s2_of[gx] = s2
                        ps_t2 = psT.tile([128, GRAN_G * 128], F16, tag="pst")
                        for r in range(RG):
                            nc.tensor.transpose(
                                out=ps_t2[:, r * 128:(r + 1) * 128],
                                in_=s2[:, r, :], identity=ident_sb[:])
                        st2 = sb.tile([128, RG * 128], F16, tag="st2")
                        nc.vector.tensor_copy(out=st2[:],
                                              in_=ps_t2[:, 0:RG * 128])
                        ps_z2 = psZ.tile([128, RG * H2], F32, tag="psz")
                        for r in range(RG):
                            nc.tensor.matmul(
                                ps_z2[:, r * H2:(r + 1) * H2],
                                lhsT=st2[:, r * 128:(r + 1) * 128],
                                rhs=a2d_g[:], start=True, stop=True)
                        nc.vector.tensor_tensor(
                            out=z2st[:, gx * RG:(gx + 1) * RG, :],
                            in0=ps_z2[:].rearrange("p (g h) -> p g h", h=H2),
                            in1=gth[:, gx * RG:(gx + 1) * RG, F2:F2 + H2],
                            op=mybir.AluOpType.add)

                    z2l = sb.tile([128, B * RG, H2], F32, tag="z2l")
                    nc.scalar.activation(z2l[:], z2st[:],
                                         mybir.ActivationFunctionType.Lrelu,
                                         alpha=NEG_SLOPE)
                    w2st = sb.tile([128, B * RG, H2], F16, tag="w2st")
                    nc.scalar.activation(w2st[:], z2l[:],
                                         mybir.ActivationFunctionType.Exp)

                    for gx in range(B):
                        g = b * B + gx
                        s2 = s2_of[gx]
                        mst2 = sb.tile([128, RG, F2], F16, tag="mst2")
                        nc.vector.tensor_tensor(
                            out=mst2[:].rearrange("p g (h c) -> p g h c",
                                                  h=H2),
                            in0=gth[:, gx * RG:(gx + 1) * RG, 0:F2]
                                .rearrange("p g (h c) -> p g h c", h=H2),
                            in1=w2st[:, gx * RG:(gx + 1) * RG, :]
                                .unsqueeze(3)
                                .to_broadcast([128, RG, H2, C2]),
                            op=mybir.AluOpType.mult)
                        ps_m2 = psM.tile([128, E1], F32, tag="psm")
                        for r in range(RG):
                            nc.tensor.matmul(ps_m2[:, 0:F2],
                                             lhsT=s2[:, r, :],
                                             rhs=mst2[:, r, :],
                                             start=(r == 0),
                                             stop=(r == RG - 1))
                            nc.tensor.matmul(
                                ps_m2[:, F2:F2 + H2], lhsT=s2[:, r, :],
                                rhs=w2st[:, gx * RG + r, :],
                                start=(r == 0), stop=(r == RG - 1))
                        if c == 0:
                            nc.vector.tensor_copy(
                                out=acc2[:, g, :], in_=ps_m2[:, 0:F2 + H2])
                        else:
                            nc.vector.tensor_tensor(
                                out=acc2[:, g, :], in0=acc2[:, g, :],
                                in1=ps_m2[:, 0:F2 + H2],
                                op=mybir.AluOpType.add)

            # ------------- finalize layer 2 + CE -------------
            for g in range(NG):
                den2 = sb.tile([128, H2], F32, tag="den")
                nc.vector.tensor_scalar_add(
                    out=den2[:], in0=acc2[:, g, F2:F2 + H2], scalar1=EPS)
                rec2 = sb.tile([128, H2], F32, tag="rec")
                nc.vector.reciprocal(out=rec2[:], in_=den2[:])
                eh = sb.tile([128, F2], F32, tag="eh")
                nc.vector.tensor_tensor(
                    out=eh[:].rearrange("p (h c) -> p h c", h=H2),
                    in0=acc2[:, g, 0:F2].rearrange("p (h c) -> p h c", h=H2),
                    in1=rec2[:].unsqueeze(2).to_broadcast([128, H2, C2]),
                    op=mybir.AluOpType.mult)
                red = sb.tile([128, C2], F32, tag="red")
                nc.vector.reduce_sum(
                    out=red[:],
                    in_=eh[:].rearrange("p (h c) -> p c h", h=H2),
                    axis=mybir.AxisListType.X)
                nc.vector.tensor_scalar_mul(out=emb2_st[:, g, :],
                                            in0=red[:], scalar1=1.0 / H2)
                nc.vector.tensor_tensor(out=emb2_st[:, g, :],
                                        in0=emb2_st[:, g, :], in1=b2_sb[:],
                                        op=mybir.AluOpType.add)
            nc.sync.dma_start(
                out=emb2_d[:].rearrange("(g p) c -> p g c", p=128),
                in_=emb2_st[:])

            mx = sb.tile([128, NG], F32, tag="mx")
            nc.vector.reduce_max(out=mx[:], in_=emb2_st[:],
                                 axis=mybir.AxisListType.X)
            sh = sb.tile([128, NG, C2], F32, tag="sh")
            nc.vector.tensor_tensor(
                out=sh[:], in0=emb2_st[:],
                in1=mx[:].unsqueeze(2).to_broadcast([128, NG, C2]),
                op=mybir.AluOpType.subtract)
            ex2 = sb.tile([128, NG, C2], F32, tag="ex2")
            nc.scalar.activation(ex2[:], sh[:],
                                 mybir.ActivationFunctionType.Exp)
            sm = sb.tile([128, NG], F32, tag="sm")
            nc.vector.reduce_sum(out=sm[:], in_=ex2[:],
                                 axis=mybir.AxisListType.X)
            lns = sb.tile([128, NG], F32, tag="lns")
            nc.scalar.activation(lns[:], sm[:],
                                 mybir.ActivationFunctionType.Ln)
            labl = sb.tile([128, NG, C2], F32, tag="labl")
            nc.vector.tensor_tensor(
                out=labl[:], in0=emb2_st[:],
                in1=oh_sb[:].rearrange("p (g c) -> p g c", c=C2),
                op=mybir.AluOpType.mult)
            labs = sb.tile([128, NG], F32, tag="labs")
            nc.vector.reduce_sum(out=labs[:], in_=labl[:],
                                 axis=mybir.AxisListType.X)
            nc.vector.tensor_tensor(out=ce_st[:], in0=mx[:], in1=lns[:],
                                    op=mybir.AluOpType.add)
            nc.vector.tensor_tensor(out=ce_st[:], in0=ce_st[:], in1=labs[:],
                                    op=mybir.AluOpType.subtract)
            nc.vector.tensor_tensor(out=ce_st[:], in0=ce_st[:],
                                    in1=mask_sb[:], op=mybir.AluOpType.mult)
            cer = sb.tile([128, 1], F32, tag="cer")
            nc.vector.reduce_sum(out=cer[:], in_=ce_st[:],
                                 axis=mybir.AxisListType.X)
            nc.sync.dma_start(out=ce_d[:], in_=cer[:])

    nc.compile()
    return nc


def postprocess(cfg, meta, results):
    emb1 = np.zeros((cfg.N, cfg.F1), np.float32)
    emb2 = np.zeros((cfg.N, cfg.C2), np.float32)
    ce_sum = 0.0
    for k in range(cfg.K):
        pr = meta["perm_rows"][k]
        pv = pr >= 0
        emb1[pr[pv]] = results[k]["emb1"][pv]
        emb2[pr[pv]] = results[k]["emb2"][pv]
        ce_sum += float(results[k]["ce"].sum())
    loss = np.float32(ce_sum / meta["mask_sum"])
    return loss, emb1, emb2


# ----------------------------------------------------------------- kernel()

_CACHE = {}

N_NODES = 100000
N_EDGES = 800000
IN_DIM = 128
HID = 16
HEADS1 = 8
HEADS2 = 10
OUT_DIM = 10


def kernel(x, edge_index, labels, train_mask, W1, a_src1, a_dst1, b1,
           W2, a_src2, a_dst2, b2):
    inputs = dict(x=x, edge_index=edge_index, labels=labels,
                  train_mask=train_mask, W1=W1, a_src1=a_src1,
                  a_dst1=a_dst1, b1=b1, W2=W2, a_src2=a_src2,
                  a_dst2=a_dst2, b2=b2)
    cfg = Cfg(N_NODES, 8, HEADS1, HID, HEADS2, OUT_DIM, IN_DIM,
              ng=112, run=256)
    in_maps, meta = preprocess(inputs, cfg)
    if "nc" not in _CACHE:
        _CACHE["nc"] = build(cfg)
    nc = _CACHE["nc"]
    res = run_bass_kernel_spmd(nc, in_maps, list(range(cfg.K)))
    loss, emb1, emb2 = postprocess(cfg, meta, res.results)
    return loss, emb1, emb2
